# revision 20
# baseline (speedup 1.0000x reference)
"""GateRecurrent2dnoind (horizontal, forward) Trainium2 kernel, v7.3.

Semantics (matching the reference):
  G1u, G2u = bilinear 2x upsample (half-pixel) of G1, G2 to (256, 256)
  g1x = G1u * X
  o = g1x; repeat 128x: o = g1x + G2u * shift_right_w(o)   (left edge replicated)

As in v6, the 128 Jacobi passes collapse into ONE sequential scan along W with
an exact depth-128 window emulation (boundary column scaled by the geometric
series of a0; excess-window corrections subtracted from d[1..16] when a0 is
near 1).

v7's big change: the per-NEFF DVE uop table is patched so that opcode 0xe5
(TensorTensorScanArith) runs WITHOUT the stock per-element bubble uop.  The
stage-1 a_flop feedback then arrives with a lag of TWO elements, so the
instruction computes two interleaved independent recurrences at ~1 cycle per
element (measured 1.06 cyc/el) instead of one recurrence at 2 cycles per
element.  All scan operands are FLAT channel-pair-interleaved streams
(strided scan APs measured 4.3 cyc/el; flat 1.06):

    stream[g*512 + 2x + c],  c in {0,1}:  s[c,x] = a[c,x]*s[c,x-1] + d[c,x]

Channel seams reset the carry because g2u column 0 is zeroed, as in v6.

PE note (measured): a matmul whose moving AP has a large innermost stride
runs at half rate (2 cyc/col).  The G tile is therefore host-packed with the
two channels of each pair column-interleaved ([pair, t, w, c]) so the moving
APs walk (j, rep, c) with strides {2, 0|4, 1} and the PSUM dst stays
contiguous while holding the (x, c)-interleaved stream order.

DMA: X and O are host-packed as per-block contiguous slabs giving 4KB/2KB
descriptor runs; G ships 2KB runs; the four upsample weight matrices ship as
one DMA.

Sharding: batch b -> core b (8 batches, 8 cores). Per core: [64, 256, 256].
"""

import numpy as np

import concourse.bacc as bacc
import concourse.bass2jax as _b2j
import concourse.bass_utils as _bu
import concourse.mybir as mybir
import concourse.tile as tile
from concourse.ap import AP
from concourse.bass_utils import run_bass_kernel_spmd
from concourse.dve_table_gen import generate_dve_tables
from concourse.dve_uop import (
    ENABLE,
    AluInp,
    AluOp,
    DelayInp,
    DveOpSpec,
    InpSel,
    OutPath,
    OutSel,
    Trigger,
    UopConfig,
)

f32 = mybir.dt.float32
f16 = mybir.dt.float16
Alu = mybir.AluOpType

NCORES = 8
C = 64          # channels per core
H = 256
W = 256
HG = 128        # G input h/w
WPAD = HG + 2   # G w + replicate pads
B = 4           # channels per block
NBLK = C // B
K = 16          # correction columns
KP = K + 1      # correction scan width per channel
KP2 = 2 * KP    # per channel-pair (interleaved)
THRESH = 0.75   # a0 mask/clamp for the correction chain
NG2C = 11       # padded G2 columns needed for g2u[0..17] (pad + cols 0..9)
FA = 2 + NG2C + 2  # aux cols per channel


# --------------------------------------------------------------------------
# DVE table patch: bubble-free TensorTensorScanArith (interleaved 2-chain)
# --------------------------------------------------------------------------

def _ttscan_nb_spec() -> DveOpSpec:
    """Stock 0xe5 program (seed/bubble/steady) minus the per-element bubble.

    Steady state issues one element per cycle; the a_flop written at stage 1
    is read back at stage 0 two elements later, giving

        out[k] = data0[k] * out[k-2] + data1[k]     (fp32 state)

    i.e. two interleaved independent recurrences at ~1 cyc/elem. op0/op1 are
    hardcoded to MULTIPLY/ADD (the instruction's dynamic op fields are
    ignored); every scan in this kernel uses (mult, add).
    """
    seed = UopConfig()
    seed.enable_input(InpSel.CONST_0, 0)
    seed.repeat_count = 1
    seed.trigger = (Trigger.COUNT, Trigger.NONE, Trigger.NONE)
    seed.next_uop = (1, 0, 0)
    seed.datapath_config[0].pass_through_alu()
    dp1 = seed.datapath_config[1].pass_through_alu()
    dp1.alu_out_a_enable = ENABLE

    bub = UopConfig()
    bub.repeat_count = 1
    bub.trigger = (Trigger.COUNT, Trigger.NONE, Trigger.NONE)
    bub.next_uop = (2, 0, 0)

    st = UopConfig()
    st.enable_input(InpSel.SRC_0, 0)
    st.enable_input(InpSel.SRC_1, 1)
    st.require_inp0 = ENABLE
    st.require_inp1 = ENABLE
    st.enable_rev_ops = ENABLE
    st.trigger = (Trigger.SRC_TENSOR_DONE, Trigger.NONE, Trigger.NONE)
    st.next_uop = (0, 0, 0)
    st.enable_output(OutSel.ALU_OUT, OutPath.WR0_LO)
    b0 = st.datapath_config[0]
    b0.enable_alu(AluOp.MULTIPLY, AluInp.PREV_ALU_OUT, AluInp.NEXT_ALU_OUT_A)
    b0.enable_delay_from_src(DelayInp.PREV_DELAY, 0)
    b1 = st.datapath_config[1]
    b1.enable_alu(AluOp.ADD, AluInp.PREV_ALU_OUT, AluInp.PREV_DELAY_0)
    b1.alu_out_a_enable = ENABLE
    for i in range(2, 8):
        st.datapath_config[i].pass_through_alu()
    return DveOpSpec(name="TTSCAN_NB", opcode=0xE5, uops=[seed, bub, st],
                     rd1_en=True)


_TABLE_CACHE: dict = {}


def _patched_dve_table_for_ops(op_names, trn_type):
    if trn_type not in _TABLE_CACHE:
        _TABLE_CACHE[trn_type] = generate_dve_tables(
            trn_type, {0xE5: _ttscan_nb_spec()})
    return dict(_TABLE_CACHE[trn_type])


_bu.dve_table_for_ops = _patched_dve_table_for_ops
_b2j.dve_table_for_ops = _patched_dve_table_for_ops


# --------------------------------------------------------------------------
# kernel body
# --------------------------------------------------------------------------

def _upsample_mats():
    """[k=in_row, m=out_row] H-upsample matrices, scaled by 0.25."""
    ue = np.zeros((HG, HG), np.float32)
    uo = np.zeros((HG, HG), np.float32)
    for m in range(HG):
        ue[m, m] += 0.25 * 0.75
        ue[max(m - 1, 0), m] += 0.25 * 0.25
        uo[m, m] += 0.25 * 0.75
        uo[min(m + 1, HG - 1), m] += 0.25 * 0.25
    return ue, uo


def _rep_ap(anchor, dims):
    """Raw AP sharing anchor's tensor/offset/partition dim, custom free dims."""
    return AP(anchor.tensor, anchor.offset, [list(anchor.ap[0])] + dims)


def _precompute(nc, ps2p, constp, weights, Auxd):
    """Boundary coefficients + correction tables (see v6).  v7 differences:
    qd/qz/qo live in channel-PAIR-interleaved layout [p, pair, x, c] so the
    correction scan runs under the interleaved-2 semantics with flat APs, and
    the qcf scale is seeded into qz (x=0 slots) instead of a post-scan
    broadcast multiply.

    Returns (d0_all [128, 2C] f16, qo_all [128, 64*KP2] f16).
    """
    C2 = 2 * C
    NPR = C2 // 2  # channel pairs across both parities
    aux = constp.tile([HG, C * FA], f16, tag="aux")
    nc.sync.dma_start(aux[:], Auxd[:])
    auxr = aux[:].rearrange("p (c f) -> p c f", c=C)

    # qz: 1.0 at each pair's x=0 slots, 0 elsewhere — no data deps, emitted
    # first on gpsimd so it never delays the DVE chain.
    qz_all = constp.tile([HG, NPR * KP2], f16, tag="qza")
    nc.gpsimd.memset(qz_all[:], 0.0)
    qzr = qz_all[:].rearrange("p (r x c) -> p r x c", r=NPR, c=2)
    nc.gpsimd.memset(qzr[:, :, 0:1, :], 1.0)
    qd_all = constp.tile([HG, NPR * KP2], f32, tag="qda")
    qdr = qd_all[:].rearrange("p (r x c) -> p r x c", r=NPR, c=2)
    nc.gpsimd.memset(qdr[:, :, 0:1, :], 0.0)

    # a0 / g1u0 for both parities via paired matmuls into one PSUM bank
    psa = ps2p.tile([HG, C2], f32, tag="ps2")
    psb = ps2p.tile([HG, C2], f32, tag="ps2")
    for pi, par in enumerate(("e", "o")):
        u1 = weights[par + "1"]
        st = pi == 0
        nc.tensor.matmul(psa[:][:, pi * C:(pi + 1) * C], u1,
                         auxr[:, :, 1], start=st, stop=not st)
        nc.tensor.matmul(psb[:][:, pi * C:(pi + 1) * C], u1,
                         auxr[:, :, 0], start=st, stop=not st)

    # -- fast track: everything the correction scan (qo) needs ------------
    a0 = constp.tile([HG, C2], f32, tag="a0")
    nc.vector.tensor_scalar_mul(a0[:], psa[:], 4.0)
    rec = constp.tile([HG, C2], f32, tag="rec")
    tr = constp.tile([HG, C2], f32, tag="tr")
    nc.vector.tensor_scalar_max(tr[:], a0[:], THRESH)
    nc.vector.reciprocal(rec[:], tr[:])
    # a0^128 by 7 squarings (independent of the geo chain below)
    p = constp.tile([HG, C2], f32, tag="p")
    nc.vector.tensor_tensor(p[:], a0[:], a0[:], Alu.mult)
    for _ in range(6):
        nc.vector.tensor_tensor(p[:], p[:], p[:], Alu.mult)
    b0 = constp.tile([HG, C2], f32, tag="b0")
    nc.vector.tensor_scalar_mul(b0[:], psb[:], 4.0)
    xc0f = constp.tile([HG, C2], f32, tag="xc0f")
    nc.vector.tensor_copy(xc0f[:][:, 0:C], auxr[:, :, FA - 2])
    nc.vector.tensor_copy(xc0f[:][:, C:C2], auxr[:, :, FA - 1])
    nc.vector.tensor_tensor(b0[:], b0[:], xc0f[:], Alu.mult)
    # qcf = b0 * mask(a0>=T) * a0^129; rq = rec*qcf scales the first factor
    # of the correction product chain (qz seeds plain 1.0)
    mask = constp.tile([HG, C2], f32, tag="mask")
    nc.vector.tensor_scalar(mask[:], a0[:], THRESH, None, Alu.is_ge)
    qcf = constp.tile([HG, C2], f32, tag="qcf")
    nc.vector.tensor_tensor(qcf[:], mask[:], p[:], Alu.mult)
    nc.vector.tensor_tensor(qcf[:], qcf[:], a0[:], Alu.mult)
    nc.vector.tensor_tensor(qcf[:], qcf[:], b0[:], Alu.mult)
    rq = constp.tile([HG, C2], f32, tag="rq")
    nc.vector.tensor_tensor(rq[:], rec[:], qcf[:], Alu.mult)

    # g2u[w] for w=0..17, all channels/parities: H-up matmul on the padded
    # head columns, W-blend as two stt per half.
    g2k18 = constp.tile([HG, C2 * 18], f32, tag="g2k18")
    g2k18r = g2k18[:].rearrange("p (c w) -> p c w", c=C2)
    for pi, par in enumerate(("e", "o")):
        u1 = weights[par + "1"]
        for half in range(2):
            ch0 = half * (C // 2)
            psh = ps2p.tile([HG, (C // 2) * NG2C], f32, tag="ps2")
            nc.tensor.matmul(
                psh[:], u1,
                auxr[:, ch0:ch0 + C // 2, 2:2 + NG2C], start=True, stop=True)
            c2s = constp.tile([HG, (C // 2) * NG2C], f32, tag="c2s")
            nc.scalar.copy(c2s[:], psh[:])
            c2r = c2s[:].rearrange("p (c w) -> p c w", c=C // 2)
            dst = g2k18r[:, pi * C + ch0:pi * C + ch0 + C // 2]
            nc.vector.scalar_tensor_tensor(
                dst[:, :, 0:17:2], c2r[:, :, 1:10], 3.0, c2r[:, :, 0:9],
                Alu.mult, Alu.add)
            nc.vector.scalar_tensor_tensor(
                dst[:, :, 1:18:2], c2r[:, :, 1:10], 3.0, c2r[:, :, 2:11],
                Alu.mult, Alu.add)

    # qd layout [p, pair, x, c]: col0 = 0 (memset above); col1 = g2u*rec*qcf
    # (carries the correction scale); cols 2..K = g2u*rec
    g2p = g2k18[:].rearrange("p (r c w) -> p r c w", r=NPR, c=2)
    g2pt = g2p.transpose([0, 1, 3, 2])  # [p, r, w, c]
    recr = rec[:].rearrange("p (r c) -> p r c", r=NPR)
    rqr = rq[:].rearrange("p (r c) -> p r c", r=NPR)
    nc.vector.tensor_tensor(
        qdr[:, :, 1:2, :], g2pt[:, :, 1:2, :],
        rqr.unsqueeze(2).to_broadcast([HG, NPR, 1, 2]), Alu.mult)
    nc.vector.tensor_tensor(
        qdr[:, :, 2:KP, :], g2pt[:, :, 2:KP, :],
        recr.unsqueeze(2).to_broadcast([HG, NPR, K - 1, 2]), Alu.mult)
    qo_all = constp.tile([HG, NPR * KP2], f16, tag="qoa")
    nc.vector.tensor_tensor_scan(
        qo_all[:], qd_all[:], qz_all[:], 0.0, Alu.mult, Alu.add)

    # -- slow track: d0_all (only needed once the block loop starts) ------
    # geo = sum_{m=0}^{127} a0^m = prod_k (1 + a0^(2^k)), k=0..6
    acc = constp.tile([HG, C2], f32, tag="acc")
    p2 = constp.tile([HG, C2], f32, tag="p2")
    t = constp.tile([HG, C2], f32, tag="t")
    nc.vector.tensor_scalar_add(acc[:], a0[:], 1.0)
    nc.vector.tensor_tensor(p2[:], a0[:], a0[:], Alu.mult)
    for _ in range(5):
        nc.vector.tensor_scalar_add(t[:], p2[:], 1.0)
        nc.vector.tensor_tensor(acc[:], acc[:], t[:], Alu.mult)
        nc.vector.tensor_tensor(p2[:], p2[:], p2[:], Alu.mult)
    nc.vector.tensor_scalar_add(t[:], p2[:], 1.0)
    nc.vector.tensor_tensor(acc[:], acc[:], t[:], Alu.mult)
    # d0_all = b0 * (1 + a0*geo)
    s0cf = constp.tile([HG, C2], f32, tag="s0cf")
    nc.vector.tensor_tensor(t[:], a0[:], acc[:], Alu.mult)
    nc.vector.tensor_scalar_add(s0cf[:], t[:], 1.0)
    d0_all = constp.tile([HG, C2], f16, tag="d0a")
    nc.vector.tensor_tensor(s0cf[:], s0cf[:], b0[:], Alu.mult)
    nc.vector.tensor_copy(d0_all[:], s0cf[:])
    return d0_all, qo_all


def _emit(nc, pools, weights, dram):
    (ginp, ps1p, ps2p, xinp, g1sp, datp, outp, constp) = pools
    Xd, Gd, Auxd, Od = dram

    d0_all, qo_all = _precompute(nc, ps2p, constp, weights, Auxd)
    qov = qo_all[:].rearrange("p (r f) -> p r f", f=KP2)

    for blk in range(NBLK):
        c0 = blk * B
        # G tile: [p, pair(2), t(2), w(130), c(2)] column-interleaved pairs
        gb = ginp.tile([HG, B * 2 * WPAD], f16, tag="gb")
        gbi = gb[:].rearrange("p (r t w c) -> p r t w c", r=2, t=2, c=2)
        nc.sync.dma_start(gb[:], Gd[:, blk])

        xb = xinp.tile([HG, 2 * 2 * W * 2], f16, tag="xb")  # (q g x c)
        nc.scalar.dma_start(xb[:], Xd[:, blk])
        # one output tile per block, DMA'd once after both parity scans
        otb = outp.tile([HG, 2 * B * W], f16, tag="ot")

        for pi, par in enumerate(("e", "o")):
            u3 = weights[par + "3"]
            u1 = weights[par + "1"]
            pstart = 0 if par == "e" else 1

            # PE: H+W upsample into PSUM, stream layout g*512 + 2x + c.
            # dst contiguous; moving walks (j, rep, c) with strides {2,0|4,1}.
            # ps1 (g1u) first so its ACT cast — the head of the longer d
            # dependency chain — starts while the ps2 matmuls still run.
            ps1 = ps1p.tile([HG, B * W], f32, tag="ps1")
            ps2 = ps2p.tile([HG, B * W], f32, tag="ps2")
            for t, ps in ((0, ps1), (1, ps2)):
                for g in range(2):
                    dst = ps[:][:, g * 2 * W:(g + 1) * 2 * W]
                    center = _rep_ap(gbi[:, g, t, 1:2, 0:1],
                                     [[2, HG], [0, 2], [1, 2]])
                    nc.tensor.matmul(dst, u3, center, start=True, stop=False)
                for g in range(2):
                    dst = ps[:][:, g * 2 * W:(g + 1) * 2 * W]
                    shift = _rep_ap(gbi[:, g, t, 0:1, 0:1],
                                    [[2, HG], [4, 2], [1, 2]])
                    nc.tensor.matmul(dst, u1, shift, start=False, stop=True)
                if t == 0:
                    # ScalarE: g1u cast to fp16, emitted BEFORE the ps2
                    # matmuls' col0 reset so the ACT queue never makes the
                    # d chain wait on ps2.
                    g1u = g1sp.tile([HG, B * W], f16, tag="g1u")
                    nc.scalar.copy(g1u[:], ps1[:])
            ps2r = ps2[:].rearrange("p (g x c) -> p g x c", g=2, c=2)
            # channel-seam reset for the scan carry (x=0 slots of both
            # chains); gpsimd cannot access PSUM, so ACT mul-by-0
            nc.scalar.mul(ps2r[:, :, 0:1, :], ps2r[:, :, 0:1, :], 0.0)

            g1v = g1u[:].rearrange("p (g f) -> p g f", g=2)
            xv = xb[:].rearrange("p (q f) -> p q f", q=2)[:, pstart] \
                .rearrange("p (g f) -> p g f", g=2)
            d = datp.tile([HG, B * W], f16, tag="d")
            dv = d[:].rearrange("p (g f) -> p g f", g=2)

            # d x=0 slots (precomputed b0*s0c), both chains
            d0v = d0_all[:, pi * C + c0:pi * C + c0 + B] \
                .rearrange("p (g c) -> p g c", g=2)
            nc.gpsimd.tensor_copy(dv[:, :, 0:2], d0v)
            # d = g1u*x in three pieces: head (DVE) feeds the gpsimd
            # correction subtract; the tail is split DVE/gpsimd to balance
            # engine busy time (gpsimd ~2ns/el vs DVE fp16 ~0.53ns/el).
            GS = 2 * 120  # stream cols of the tail handled by gpsimd
            nc.vector.tensor_tensor(
                dv[:, :, 2:2 * (K + 2)], g1v[:, :, 2:2 * (K + 2)],
                xv[:, :, 2:2 * (K + 2)], Alu.mult)
            pr0 = pi * (C // 2) + blk * 2
            nc.gpsimd.tensor_tensor(
                dv[:, :, 2:KP2], dv[:, :, 2:KP2],
                qov[:, pr0:pr0 + 2, 2:KP2], Alu.subtract)
            nc.gpsimd.tensor_tensor(
                dv[:, :, 2 * (K + 2):2 * (K + 2) + GS],
                g1v[:, :, 2 * (K + 2):2 * (K + 2) + GS],
                xv[:, :, 2 * (K + 2):2 * (K + 2) + GS], Alu.mult)
            nc.vector.tensor_tensor(
                dv[:, :, 2 * (K + 2) + GS:], g1v[:, :, 2 * (K + 2) + GS:],
                xv[:, :, 2 * (K + 2) + GS:], Alu.mult)

            # main scan (interleaved-2): s[c,x] = g2u[c,x]*s[c,x-1] + d[c,x]
            nc.vector.tensor_tensor_scan(
                otb[:][:, pstart * B * W:(pstart + 1) * B * W],
                ps2[:], d[:], 0.0, Alu.mult, Alu.add)
        nc.sync.dma_start(Od[:, blk], otb[:])


def build():
    nc = bacc.Bacc("TRN2", target_bir_lowering=False, debug=False,
                   num_devices=NCORES)
    Xd = nc.dram_tensor("X", [HG, NBLK, 2 * 2 * W * 2], f16,
                        kind="ExternalInput")
    Gd = nc.dram_tensor("G12", [HG, NBLK, B * 2 * WPAD], f16,
                        kind="ExternalInput")
    Auxd = nc.dram_tensor("AUX", [HG, C * FA], f16, kind="ExternalInput")
    Ud = nc.dram_tensor("U", [HG, 4 * HG], f16, kind="ExternalInput")
    Od = nc.dram_tensor("O", [HG, NBLK, 2 * B * W], f16,
                        kind="ExternalOutput")

    with tile.TileContext(nc) as tc:
        with (
            tc.tile_pool(name="const", bufs=1) as constp,
            tc.tile_pool(name="gin", bufs=8) as ginp,
            tc.tile_pool(name="ps1", bufs=2, space="PSUM") as ps1p,
            tc.tile_pool(name="ps2", bufs=2, space="PSUM") as ps2p,
            tc.tile_pool(name="xin", bufs=6) as xinp,
            tc.tile_pool(name="g1s", bufs=6) as g1sp,
            tc.tile_pool(name="dat", bufs=6) as datp,
            tc.tile_pool(name="out", bufs=6) as outp,
        ):
            wt = constp.tile([HG, 4 * HG], f16, tag="uw")
            nc.scalar.dma_start(wt[:], Ud[:])
            weights = {n: wt[:][:, i * HG:(i + 1) * HG]
                       for i, n in enumerate(("e3", "e1", "o3", "o1"))}
            # PE warm-up: ~3.5us of dummy matmuls (weights tile as moving
            # data) so the HAM clock gate opens to 2.4 GHz before the real
            # upsample matmuls start.
            for wrm in range(8):
                pw = ps1p.tile([HG, B * W], f32, tag="ps1")
                mv = _rep_ap(wt[:][:, 0:1], [[1, HG], [0, 4]])
                nc.tensor.matmul(pw[:][:, 0:4 * HG], weights["e3"], mv,
                                 start=True, stop=True)
            pools = (ginp, ps1p, ps2p, xinp, g1sp, datp, outp, constp)
            _emit(nc, pools, weights, (Xd, Gd, Auxd, Od))

    nc.compile()
    nc.m.ant_custom_dve_ops = ["TTSCAN_NB"]
    return nc


_NC = None


def kernel(X, G1, G2, G3=None, **_):
    global _NC
    if _NC is None:
        _NC = build()
    ue, uo = _upsample_mats()
    wcat = np.concatenate(
        [(3.0 * ue), ue, (3.0 * uo), uo], axis=1).astype(np.float16)

    def pad(G):
        return np.concatenate([G[..., :1], G, G[..., -1:]], axis=-1)

    Xh = np.asarray(X).astype(np.float16)
    # X: [C,H,W] -> [h2, blk, q, g, x, c] flat per core
    Xp = Xh.reshape(NCORES, NBLK, 2, 2, HG, 2, W)   # k blk g c h2 q x
    Xp = np.ascontiguousarray(Xp.transpose(0, 4, 1, 5, 2, 6, 3))
    Xp = Xp.reshape(NCORES, HG, NBLK, 2 * 2 * W * 2)

    pg1 = pad(np.asarray(G1)).astype(np.float16)    # [k, C, HG, WPAD]
    pg2 = pad(np.asarray(G2)).astype(np.float16)
    # G: [k, hg, pair, t, w, c] (channels of each pair column-interleaved)
    G12h = np.stack([pg1, pg2], axis=2)             # [k, C, t, HG, WPAD]
    G12h = G12h.reshape(NCORES, C // 2, 2, 2, HG, WPAD)  # k pr cc t hg w
    G12p = np.ascontiguousarray(G12h.transpose(0, 4, 1, 3, 5, 2))
    G12p = G12p.reshape(NCORES, HG, NBLK, B * 2 * WPAD)

    # host-packed aux: per (h-row, channel): G1 col0, G2 col0, padded G2
    # head cols 0..NG2C-1, X col0 (even rows), X col0 (odd rows)
    aux = np.empty((NCORES, HG, C, FA), np.float16)
    aux[..., 0] = pg1[:, :, :, 1].transpose(0, 2, 1)
    aux[..., 1] = pg2[:, :, :, 1].transpose(0, 2, 1)
    aux[..., 2:2 + NG2C] = pg2[:, :, :, 0:NG2C].transpose(0, 2, 1, 3)
    aux[..., FA - 2] = Xh[:, :, 0::2, 0].transpose(0, 2, 1)
    aux[..., FA - 1] = Xh[:, :, 1::2, 0].transpose(0, 2, 1)
    aux = aux.reshape(NCORES, HG, C * FA)

    in_maps = [
        {"X": np.ascontiguousarray(Xp[k]), "G12": G12p[k],
         "AUX": np.ascontiguousarray(aux[k]), "U": wcat}
        for k in range(NCORES)
    ]
    res = run_bass_kernel_spmd(_NC, in_maps, list(range(NCORES)))
    kernel.last_result = res
    outs = []
    for k in range(NCORES):
        O = res.results[k]["O"].reshape(HG, NBLK, 2, 2, W, 2)
        # [h2, blk, q, g, x, c] -> [c_full, h, w]
        outs.append(O.transpose(1, 3, 5, 0, 2, 4).reshape(C, H, W))
    return np.stack(outs).astype(np.float32)
